# revision 30
# baseline (speedup 1.0000x reference)
"""Trainium2 Bass kernel for nn_AttentionLayer_66949950210666.

Cross-attention layer: q from decoder_hs, k/v from encoder_hs,
16 heads, D=1024, S=2048, B=2, fp32.

Sharding (8 cores): core c = (b, r) with b = c // 4, r = c % 4.
Each core handles batch b and heads [4r, 4r+4) (o-dims [256r, 256r+256)).
Device-side, everything lives in a "transposed world":
  QT[o, s], KT[o, s]  (o on partitions)  and V[s, o] (s on partitions),
so the attention works without any on-chip transposes:
  ST[k, q]   = KT_h^T-contract-d  (matmul lhsT=KT slice, rhs=QT slice, K=64)
  PT[k, q]   = exp(ST / 8)                       (ScalarE, no max-subtract;
                                                  |S| ~ N(0,1), fp32-safe)
  YuT[d, q]  = sum_k Vaug[k, d|ones] PT[k, q]    (K=128 matmul; the extra
                                                  "ones" column makes row 64
                                                  the softmax denominator)
  YT = YuT * recip(denom)  broadcast across partitions via a tiny K=1 matmul
Final projection after an AllToAll inside each 4-core batch group: each
core gathers the full Y^T[j=1024, its 512 s-columns] and computes
ZT[o, s_r] = Wp.T-contract-j + c, where c = Wp @ bv + bp is folded on the
host (this removes the need for any free-dim bias broadcast on device).

Host returns out[b, 512r:512(r+1), :] = ZT.T per core.
"""

import sys

sys.path.insert(0, "/opt/trn_rl_repo")

import ml_dtypes
import numpy as np

import bass_rust as _bass_rust

import concourse.bass as bass
import concourse.mybir as mybir
import concourse.tile as tile
from concourse import bacc
from concourse.bass_utils import run_bass_kernel_spmd

F32 = mybir.dt.float32
BF16 = mybir.dt.bfloat16

# The greedy ACT-table chooser would ping-pong between exp_and_others and
# natural_log for our Exp/Ln mix (one ~2.7us table load per switch).  Hide
# Exp/Ln from the single-function sets so every activation resolves to
# natural_log_exp_and_others.  Only the membership sets are changed — dict
# order/length (the act_func_set_id space) is untouched.
import concourse.hw_specs as _hw_specs
from concourse import bacc as _bacc_mod

_orig_get_tables = _hw_specs.get_activation_tables


def _patched_get_tables(arch):
    t = {k: set(v) for k, v in _orig_get_tables(arch).items()}
    if "natural_log_exp_and_others" in t:
        for name, fns in t.items():
            if name != "natural_log_exp_and_others":
                fns.discard(mybir.ActivationFunctionType.Exp)
                fns.discard(mybir.ActivationFunctionType.Ln)
    return t


_bacc_mod.get_activation_tables = _patched_get_tables

B, S, D, H, HD = 2, 2048, 1024, 16, 64
NCORES = 8
GROUPS = [[0, 1, 2, 3], [4, 5, 6, 7]]
OL = 256          # local output dims (4 heads x 64)
SB = S // 8       # 256: s-slice per core after the 8-way AllToAll
NST = S // 512    # 4 s-tiles of 512
NDC = D // 128    # 8 contraction chunks
NKT = S // 128    # 16 k-tiles
SCALE = 0.125     # 1/sqrt(HD)


def build_nc():
    nc = bacc.Bacc(None, num_devices=NCORES, target_bir_lowering=False)

    xdT = nc.declare_dram_parameter("xdT", [D, S], BF16, isOutput=False)
    xeT = nc.declare_dram_parameter("xeT", [D, S], BF16, isOutput=False)
    wqT = nc.declare_dram_parameter("wqT", [D, OL], BF16, isOutput=False)
    wkT = nc.declare_dram_parameter("wkT", [D, OL], BF16, isOutput=False)
    wvT = nc.declare_dram_parameter("wvT", [D, OL], BF16, isOutput=False)
    wpT = nc.declare_dram_parameter("wpT", [D, D], BF16, isOutput=False)
    bqP = nc.declare_dram_parameter("bq", [2, 128], F32, isOutput=False)
    bkP = nc.declare_dram_parameter("bk", [2, 128], F32, isOutput=False)
    cbP = nc.declare_dram_parameter("cb", [8, 128], F32, isOutput=False)
    ztO = nc.declare_dram_parameter("zT", [2, D, SB], F32, isOutput=True)

    with tile.TileContext(nc) as tc:
        with (
            tc.tile_pool(name="const", bufs=1) as const,
            tc.tile_pool(name="big", bufs=1) as big,
            tc.tile_pool(name="xp", bufs=8) as xp,
            tc.tile_pool(name="dram", bufs=1, space="DRAM") as dram,
        ):
            # ---- constants / weights resident in SBUF ----
            wq_s = const.tile([128, NDC, OL], BF16)
            wk_s = const.tile([128, NDC, OL], BF16)
            wv_s = const.tile([128, NDC, OL], BF16)
            nc.sync.dma_start(wq_s[:], wqT.rearrange("(dc p) o -> p dc o", p=128))
            nc.sync.dma_start(wk_s[:], wkT.rearrange("(dc p) o -> p dc o", p=128))
            nc.sync.dma_start(wv_s[:], wvT.rearrange("(dc p) o -> p dc o", p=128))
            bq_s = const.tile([128, 2], F32)
            bk_s = const.tile([128, 2], F32)
            cb_s = const.tile([128, 8], F32)
            nc.sync.dma_start(bq_s[:], bqP.rearrange("a p -> p a"))
            nc.sync.dma_start(bk_s[:], bkP.rearrange("a p -> p a"))
            nc.sync.dma_start(cb_s[:], cbP.rearrange("a p -> p a"))
            ones_sb = const.tile([128, 64], BF16)
            nc.vector.memset(ones_sb[:], 1.0)

            # persistent activations
            QT = [big.tile([128, S], BF16, tag=f"QT{i}", name=f"QT{i}") for i in range(2)]
            KT = [big.tile([128, S], BF16, tag=f"KT{i}", name=f"KT{i}") for i in range(2)]
            # V augmented with a ones column per head: [k-part, kt, h, 65]
            vaug = big.tile([128, NKT, 4, 65], BF16, tag="vaug")
            nc.vector.memset(vaug[:, :, :, 64:65], 1.0)

            # ---- phase B: K and V projections only (Q is folded into the
            # attention loop so exp can start ~30us earlier) ----
            with tc.tile_pool(name="bps", bufs=1, space="PSUM") as bps:
                for st in range(NST):
                    ssl = slice(st * 512, (st + 1) * 512)
                    kps = [bps.tile([128, 512], F32, tag=f"k{oc}", name=f"kps{oc}", bufs=2) for oc in range(2)]
                    vps = [bps.tile([128, 256], F32, tag=f"v{u}", name=f"vps{u}") for u in range(4)]
                    xts = []
                    for dch in range(NDC // 2):
                        xe_t = xp.tile([128, 2, 512], BF16, tag="xe")
                        nc.sync.dma_start(
                            xe_t[:], xeT[dch * 256:(dch + 1) * 256, ssl]
                            .rearrange("(two p) s -> p two s", p=128))
                        xts.append(xe_t)
                    for dc in range(NDC):
                        xe_t = xts[dc // 2][:, dc % 2, :]
                        st_ = dc == 0
                        sp_ = dc == NDC - 1
                        for oc in range(2):
                            nc.tensor.matmul(
                                kps[oc][:], (wk_s[:, dc, oc * 128:(oc + 1) * 128]),
                                xe_t, start=st_, stop=sp_)
                        for u in range(4):
                            nc.tensor.matmul(
                                vps[u][:], xe_t[:, u * 128:(u + 1) * 128],
                                (wv_s[:, dc, :]), start=st_, stop=sp_)
                    for oc in range(2):
                        nc.vector.tensor_scalar_add(
                            KT[oc][:, ssl], kps[oc][:], bk_s[:, oc:oc + 1])
                    for u in range(4):
                        kt = st * 4 + u
                        # ScalarE is idle here; keep DVE for the bias adds
                        nc.scalar.copy(
                            vaug[:, kt, :, 0:64],
                            vps[u][:].rearrange("p (h d) -> p h d", h=4))

            # prefetch Wp while attention runs
            wp_s = const.tile([128, NDC, D], BF16)
            nc.sync.dma_start(wp_s[:], wpT.rearrange("(jc p) o -> p jc o", p=128))

            # ---- phase C: attention ----
            # One [128,1024] two-bank PSUM "super" holds S^T for both heads
            # of a k-tile; a single Exp processes it (amortizes the ~350cyc
            # fixed ACT cost).  PV accumulates [V_h|ones] so row 64 of yu is
            # the softmax denominator.  Y^T streams to DRAM per (pair, qt)
            # and each pair's AllToAll launches as soon as the pair is done,
            # hiding the first collective under the second pair's compute.
            ytgEO = [const.tile([128, 2, NDC // 2, SB], BF16, name=f"ytg{p}")
                     for p in range(2)]
            ydramP = [dram.tile([8, 128, SB], BF16, name=f"ydram{p}") for p in range(2)]
            ygathP = [dram.tile([8, 128, SB], BF16, name=f"ygath{p}") for p in range(2)]
            with (
                tc.tile_pool(name="stp", bufs=2, space="PSUM") as stp,
                tc.tile_pool(name="yup", bufs=3, space="PSUM") as yup,
                tc.tile_pool(name="aux", bufs=1, space="PSUM") as auxp,
                tc.tile_pool(name="pt", bufs=4) as ptp,
                tc.tile_pool(name="ep", bufs=4) as ep,
            ):
                def emit_q(st):
                    # lazy Q projection for s-block st, interleaved into the
                    # attention stream (PE has spare capacity under ACT pacing)
                    ssl = slice(st * 512, (st + 1) * 512)
                    xts = []
                    for dch in range(NDC // 2):
                        xd_t = xp.tile([128, 2, 512], BF16, tag="xd", name="xd_t")
                        nc.sync.dma_start(
                            xd_t[:], xdT[dch * 256:(dch + 1) * 256, ssl]
                            .rearrange("(two p) s -> p two s", p=128))
                        xts.append(xd_t)
                    for oc in range(2):
                        qps = auxp.tile([128, 512], F32, tag="aux", name="qps")
                        for dc in range(NDC):
                            nc.tensor.matmul(
                                qps[:], (wq_s[:, dc, oc * 128:(oc + 1) * 128]),
                                xts[dc // 2][:, dc % 2, :],
                                start=(dc == 0), stop=(dc == NDC - 1))
                        nc.vector.tensor_scalar_add(
                            QT[oc][:, ssl], qps[:], bq_s[:, oc:oc + 1])

                def finish_qt(pair, qt, yufs, anchor):
                    # deferred normalize+store; the reciprocal runs on DVE in
                    # parallel with the exp stream, and the tiny R matmuls are
                    # pinned behind `anchor` so the scheduler cannot hoist
                    # them into a head-of-line block on the reciprocal
                    for hh in range(2):
                        yuf = yufs[hh]
                        rrec = ep.tile([128, 512], BF16, tag="rrec", name="rrec")
                        with nc.allow_low_precision(reason="bf16 softmax recip"):
                            nc.vector.reciprocal(rrec[64:65, :], yuf[64:65, :])
                        rps = auxp.tile([64, 512], F32, tag="aux", name="rps")
                        rmm = nc.tensor.matmul(
                            rps[:], ones_sb[64:65, :], rrec[64:65, :],
                            start=True, stop=True)
                        _bass_rust.add_dep_helper(
                            rmm.ins, anchor.ins, sync=False,
                            reason="pin R-matmul after current attention MMs")
                        yst = ep.tile([64, 512], BF16, tag="yst", name="yst")
                        nc.vector.tensor_mul(yst[:], yuf[0:64, :], rps[:])
                        nc.sync.dma_start(
                            ydramP[pair][2 * qt:2 * qt + 2,
                                         64 * hh:64 * (hh + 1), :]
                            .rearrange("s j q -> j s q"),
                            yst[:].rearrange("j (s q) -> j s q", s=2))

                emit_q(0)
                for pair in range(2):
                    pending = None
                    for qt in range(NST):
                        qsl = slice(qt * 512, (qt + 1) * 512)
                        yu = [yup.tile([128, 512], F32, tag="yu", name=f"yu{hh}") for hh in range(2)]
                        for kt in range(NKT):
                            ksl = slice(kt * 128, (kt + 1) * 128)
                            sps = stp.tile([128, 1024], F32, tag="st")
                            for hh in range(2):
                                psl = slice(64 * hh, 64 * (hh + 1))
                                nc.tensor.matmul(
                                    sps[:, 512 * hh:512 * (hh + 1)],
                                    KT[pair][psl, ksl], QT[pair][psl, qsl],
                                    start=True, stop=True)
                            pt_t = ptp.tile([128, 1024], BF16, tag="pt")
                            nc.scalar.activation(
                                pt_t[:], sps[:],
                                mybir.ActivationFunctionType.Exp, scale=SCALE)
                            for hh in range(2):
                                h = 2 * pair + hh
                                last_pv = nc.tensor.matmul(
                                    yu[hh][0:65, :], vaug[:, kt, h, :],
                                    pt_t[:, 512 * hh:512 * (hh + 1)],
                                    start=(kt == 0), stop=(kt == NKT - 1))
                            if kt == 5 and pending is not None:
                                finish_qt(pair, *pending, anchor=last_pv)
                                pending = None
                            if kt == 7 and pair == 0 and qt < NST - 1:
                                emit_q(qt + 1)
                        # evacuate PSUM immediately (DVE only, no PE ops)
                        yufs = []
                        for hh in range(2):
                            yuf = ep.tile([65, 512], F32, tag="yuf", name="yuf")
                            nc.vector.tensor_copy(yuf[:], yu[hh][0:65, :])
                            yufs.append(yuf)
                        pending = (qt, yufs)
                    finish_qt(pair, *pending, anchor=last_pv)
                    # pair's AllToAll: pair 0's hides under pair 1's compute
                    nc.gpsimd.collective_compute(
                        "AllToAll", mybir.AluOpType.bypass,
                        replica_groups=[list(range(NCORES))],
                        ins=[ydramP[pair].opt()], outs=[ygathP[pair].opt()])
                    # ygathP[p][4*bb+g] = Y^T rows for global j chunk 2g+p
                    nc.sync.dma_start(
                        ytgEO[pair][:],
                        ygathP[pair][:].rearrange("(bb g) j q -> j bb g q", bb=2))

            # ---- phase D: output projection, two passes so the pair-0 half
            # runs during the second AllToAll (also keeps the PE warm) ----
            zacc = const.tile([128, 2, NDC, SB], F32)
            with (
                tc.tile_pool(name="zps", bufs=4, space="PSUM") as zpsp,
                tc.tile_pool(name="zt", bufs=4) as ztp,
            ):
                for bb in range(2):
                    for oc in range(NDC):
                        zps = zpsp.tile([128, SB], F32, tag="z", name="zps")
                        for g in range(4):
                            nc.tensor.matmul(
                                zps[:], (wp_s[:, 2 * g, oc * 128:(oc + 1) * 128]),
                                (ytgEO[0][:, bb, g, :]),
                                start=(g == 0), stop=(g == 3))
                        nc.vector.tensor_copy(zacc[:, bb, oc, :], zps[:])
                for bb in range(2):
                    for oc in range(NDC):
                        zps = zpsp.tile([128, SB], F32, tag="z", name="zps2")
                        for g in range(4):
                            nc.tensor.matmul(
                                zps[:], (wp_s[:, 2 * g + 1, oc * 128:(oc + 1) * 128]),
                                (ytgEO[1][:, bb, g, :]),
                                start=(g == 0), stop=(g == 3))
                        zt_t = ztp.tile([128, SB], F32, tag="zt", name="zt_t")
                        nc.vector.scalar_tensor_tensor(
                            zt_t[:], zps[:], cb_s[:, oc:oc + 1],
                            zacc[:, bb, oc, :],
                            op0=mybir.AluOpType.add, op1=mybir.AluOpType.add)
                        nc.sync.dma_start(
                            ztO[bb, oc * 128:(oc + 1) * 128, :], zt_t[:])

    nc.compile()
    return nc


def make_in_maps(decoder_hs, encoder_hs, Wq, bq, Wk, bk, Wv, bv, Wp, bp):
    dh = np.ascontiguousarray(np.asarray(decoder_hs, np.float32))
    eh = np.ascontiguousarray(np.asarray(encoder_hs, np.float32))
    Wq, Wk, Wv, Wp = (np.asarray(a, np.float32) for a in (Wq, Wk, Wv, Wp))
    bq, bk, bv, bp = (np.asarray(a, np.float32) for a in (bq, bk, bv, bp))
    c = (Wp @ bv + bp).astype(np.float32)
    bf = ml_dtypes.bfloat16
    wpT = np.ascontiguousarray(Wp.T).astype(bf)
    xdT = [np.ascontiguousarray(dh[b].T).astype(bf) for b in range(B)]
    xeT = [np.ascontiguousarray(eh[b].T).astype(bf) for b in range(B)]
    in_maps = []
    for core in range(NCORES):
        b, r = divmod(core, 4)
        sl = slice(OL * r, OL * (r + 1))
        in_maps.append({
            "xdT": xdT[b],
            "xeT": xeT[b],
            "wqT": np.ascontiguousarray(Wq[sl].T).astype(bf),
            "wkT": np.ascontiguousarray(Wk[sl].T).astype(bf),
            "wvT": np.ascontiguousarray(Wv[sl].T).astype(bf),
            "wpT": wpT,
            "bq": np.ascontiguousarray(bq[sl].reshape(2, 128)),
            "bk": np.ascontiguousarray(bk[sl].reshape(2, 128)),
            "cb": np.ascontiguousarray(c.reshape(8, 128)),
        })
    return in_maps


def assemble_output(results):
    out = np.empty((B, S, D), np.float32)
    for core in range(NCORES):
        zT = np.asarray(results[core]["zT"])  # [2, 1024, 256]
        for b in range(B):
            out[b, SB * core:SB * (core + 1), :] = zT[b].T
    return out


_NC = None


def kernel(**inputs):
    global _NC
    if _NC is None:
        _NC = build_nc()
    in_maps = make_in_maps(**inputs)
    res = run_bass_kernel_spmd(_NC, in_maps, list(range(NCORES)))
    return assemble_output(res.results)


if __name__ == "__main__":
    nc = build_nc()
    print("built ok")
